# revision 32
# baseline (speedup 1.0000x reference)
"""Trainium2 Bass kernel for CausalSelfAttention (B=2, T=2048, D=1024, H=16).

Sharding (8 cores): Megatron-style tensor parallel. Core c owns heads
{2c, 2c+1}: column-parallel c_attn (384 of 3072 output features),
full attention for its 2 heads x 2 batches, row-parallel c_proj
(128 of 1024 contraction rows). Host sums the 8 partial outputs and
adds b_proj.

Device algorithm (per core), all matmuls bf16, softmax f32:
  1. qkv^T = Wslice^T @ x^T   -- x^T arrives pre-transposed bf16 from host.
     q^T, k^T stay in SBUF; v^T tiles are PE-transposed (identity matmul)
     into natural [k, d] layout with a constant ones column appended
     (softmax denominator rides the PV matmul as row 64).
  2. Attention in the TRANSPOSED orientation, one 128-key tile at a time:
     S^T[k, q] = k^T.T @ q^T into the two halves of a [128, 2, 512] f32
     PSUM tile (2 banks); the two heads' S matmuls run CONCURRENTLY in
     the PE array (row groups 0-63 / 64-127 via auto tile_position) and
     write different banks. ONE exp per k-tile on ACT covers both heads
     straight out of PSUM, with a per-partition bias column that carries
     both the -10 shift (cancels in the softmax ratio; logits are O(1),
     so no max-subtraction) and the additive key-mask (-50 for masked
     keys, exp -> 0; the bias depends only on the k-tile so both heads
     share it). Causal: skip invalid column ranges + an upper-triangular
     multiplicative mask on diagonal blocks (DVE). The 2-slot two-bank
     PSUM rotation (tag shared with the padded qkv-phase accumulators)
     lets S of tile j+1 overlap exp of tile j.
  3. PV: out[65, q] = [v_h | ones].T @ P^T accumulated over k-tiles.
     Row 64 is the denominator (masked keys contribute exp(-60)~0).
     rq = qmask / (denom + eps) broadcast across partitions on GpSimd,
     multiplied into y^T on DVE.
  4. out = y^T.T @ Wproj_rows -> f32 PSUM, DVE cast to bf16 SBUF, DMA.
     Host sums the 8 partials + b_proj.
"""

import functools

import numpy as np
import ml_dtypes

import concourse.bass as bass
import concourse.mybir as mybir
import concourse.tile as tile
from concourse import bacc
from concourse.bass_utils import run_bass_kernel_spmd
from concourse.masks import make_upper_triangular, make_identity

BF16 = mybir.dt.bfloat16
F32 = mybir.dt.float32
AF = mybir.ActivationFunctionType
OP = mybir.AluOpType

B, T, D, NH = 2, 2048, 1024, 16
DH = 64                  # head dim
HPC = 2                  # heads per core
NCORES = 8
TT = B * T               # 4096 total tokens
P = 128
KC = D // P              # 8 contraction tiles for qkv
SPAN = 512               # q-span processed per softmax pass
NSP = T // SPAN          # 4 spans per batch
NKT = T // P             # 16 k-tiles per batch
QSCALE = 1.0 / np.sqrt(DH)
ESHIFT = -10.0           # constant exp shift; cancels in softmax ratio
MASKP = -50.0            # additive key-mask penalty (pre-exp)
VW = 2 * DH + 2          # v_nat width: [v_h0 | 1 | v_h1 | 1]
FP8_QKV = False          # fp8e4m3 DoubleRow qkv: fails rel-err gate (~4e-2)
FP8_S = False            # fp8 DoubleRow S: no gain (stream-bound, K=64 fits
                         # one matmul already) and costs accuracy (1.5e-2)
K2 = KC // 2             # DoubleRow k-tile pairs
FP8 = mybir.dt.float8e4


def build():
    nc = bacc.Bacc(None)

    if FP8_QKV:
        xT = nc.dram_tensor("xT", [P, K2, 2, TT], FP8, kind="ExternalInput")
        wqkv = nc.dram_tensor("wqkv", [P, K2, 2, 3 * P], FP8,
                              kind="ExternalInput")
    else:
        xT = nc.dram_tensor("xT", [D, TT], BF16, kind="ExternalInput")
        wqkv = nc.dram_tensor("wqkv", [P, KC, 3 * P], BF16, kind="ExternalInput")
    bqkv = nc.dram_tensor("bqkv", [P, 3], F32, kind="ExternalInput")
    wproj = nc.dram_tensor("wproj", [P, D], BF16, kind="ExternalInput")
    mrowinv = nc.dram_tensor("mrowinv", [1, TT], F32, kind="ExternalInput")
    mbias = nc.dram_tensor("mbias", [P, B, NKT], F32, kind="ExternalInput")
    out = nc.dram_tensor("out", [TT, D], BF16, kind="ExternalOutput")

    with tile.TileContext(nc) as tc:
        with (
            tc.tile_pool(name="singles", bufs=1) as singles,
            tc.tile_pool(name="stage", bufs=3) as stage,
            tc.tile_pool(name="pt", bufs=6) as ptp,
            tc.tile_pool(name="rows", bufs=4) as rows,
            tc.tile_pool(name="outs", bufs=4) as outs,
            # 2 two-bank slots shared by qkv accumulators and paired S^T tiles
            tc.tile_pool(name="psA", bufs=2, space="PSUM") as psA,
            # 4 one-bank slots shared by pv accumulators, proj out, transposes
            tc.tile_pool(name="psB", bufs=4, space="PSUM") as psB,
        ):
            # ---- constants / weights (small DMAs first: casts block on them) ----
            if FP8_QKV:
                wqkv_sb = singles.tile([P, K2, 2, 3 * P], FP8)
                nc.sync.dma_start(out=wqkv_sb, in_=wqkv[:, :, :, :])
            else:
                wqkv_sb = singles.tile([P, KC, 3 * P], BF16)
                nc.sync.dma_start(out=wqkv_sb, in_=wqkv[:, :, :])
            bqkv_sb = singles.tile([P, 3], F32)
            nc.sync.dma_start(out=bqkv_sb, in_=bqkv[:, :])
            if FP8_QKV:
                xT_sb = singles.tile([P, K2, 2, TT], FP8)

                def dma_x(k, tsl):
                    nc.sync.dma_start(out=xT_sb[:, k, :, tsl],
                                      in_=xT[:, k, :, tsl])
                nkx = K2
            else:
                xT_sb = singles.tile([P, KC, TT], BF16)

                def dma_x(k, tsl):
                    nc.sync.dma_start(out=xT_sb[:, k, tsl],
                                      in_=xT[k * P:(k + 1) * P, tsl])
                nkx = KC
            # first qkv group only needs [all k, n2=0]: fetch those before the
            # small attention-phase constants so the PE starts earlier
            for k in range(nkx):
                dma_x(k, slice(0, 1024))
            wproj_sb = singles.tile([P, D], BF16)
            nc.sync.dma_start(out=wproj_sb, in_=wproj[:, :])
            mrowinv_sb = singles.tile([1, TT], F32)
            nc.sync.dma_start(out=mrowinv_sb, in_=mrowinv[:, :])
            mbias_sb = singles.tile([P, B, NKT], F32)
            nc.sync.dma_start(out=mbias_sb, in_=mbias[:, :, :])
            for n2 in range(1, TT // 1024):
                for k in range(nkx):
                    dma_x(k, slice(n2 * 1024, (n2 + 1) * 1024))

            ut_sb = singles.tile([P, P], BF16)  # keep q >= k
            make_upper_triangular(nc, ut_sb, val=1.0, diag=True)
            ident = singles.tile([P, P], BF16)
            make_identity(nc, ident)

            QKDT = FP8 if FP8_S else BF16
            qT_sb = singles.tile([P, TT], QKDT)   # rows: h0 d0..63 | h1 d0..63
            kT_sb = singles.tile([P, TT], QKDT)
            if FP8_S:
                # DoubleRow layout: [32h + d%32, d//32, t]
                qT8_sb = singles.tile([64, 2, TT], FP8)
                kT8_sb = singles.tile([64, 2, TT], FP8)
            yT_sb = singles.tile([P, TT], BF16)
            v_nat = singles.tile([P, NKT * B, VW], BF16)
            # denominator ones columns (64 and 129), constant across tiles
            nc.vector.memset(v_nat[:, :, DH:DH + 1], 1.0)
            nc.vector.memset(v_nat[:, :, VW - 1:VW], 1.0)

            # ---- phase 1: qkv^T = W^T @ x^T ----
            # [128 feat, 512 t] accumulators; n2 0-1 (batch 0) up front, the
            # n2 2-3 groups are interleaved into batch-0's attention loop as
            # tensor filler for the ACT-saturation stalls at span ends
            def emit_qkv(n2, m, h2):
                        pq = psA.tile([P, 512], F32, tag="b1", name="pq",
                                      padded_shape=[P, 1024])
                        t0 = n2 * 1024 + h2 * 512
                        if FP8_QKV:
                            for k2 in range(K2):
                                nc.tensor.matmul(
                                    pq[:],
                                    wqkv_sb[:, k2, :, m * P:(m + 1) * P],
                                    xT_sb[:, k2, :, t0:t0 + 512],
                                    start=(k2 == 0), stop=(k2 == K2 - 1),
                                    perf_mode=mybir.MatmulPerfMode.DoubleRow,
                                )
                        else:
                            for k in range(KC):
                                nc.tensor.matmul(
                                    pq[:],
                                    wqkv_sb[:, k, m * P:(m + 1) * P],
                                    xT_sb[:, k, t0:t0 + 512],
                                    start=(k == 0), stop=(k == KC - 1),
                                )
                        tcols = slice(t0, t0 + 512)
                        if m == 0:
                            nc.scalar.activation(
                                qT_sb[:, tcols], pq[:], AF.Identity,
                                bias=bqkv_sb[:, 0:1], scale=QSCALE)
                        elif m == 1:
                            nc.scalar.activation(
                                kT_sb[:, tcols], pq[:], AF.Identity,
                                bias=bqkv_sb[:, 1:2], scale=1.0)
                        else:
                            vst = stage.tile([P, 512], BF16, tag="vst")
                            nc.scalar.activation(
                                vst[:], pq[:], AF.Identity,
                                bias=bqkv_sb[:, 2:3], scale=1.0)
                            # phase 2: v natural [k, d] via PE transpose
                            for jj in range(512 // P):
                                j32 = n2 * 8 + h2 * 4 + jj
                                vtp = psB.tile([P, P], BF16, tag="pv")
                                nc.tensor.transpose(
                                    vtp[:], vst[:, jj * P:(jj + 1) * P], ident[:])
                                nc.vector.tensor_copy(
                                    out=v_nat[:, j32, 0:DH], in_=vtp[:, 0:DH])
                                nc.vector.tensor_copy(
                                    out=v_nat[:, j32, DH + 1:2 * DH + 1],
                                    in_=vtp[:, DH:2 * DH])
            qkv_todo = [(n2, m, h2) for n2 in range(TT // 1024)
                        for m in range(3) for h2 in range(2)]
            while qkv_todo and qkv_todo[0][0] < 2:
                emit_qkv(*qkv_todo.pop(0))
            if FP8_S:
                # partition shuffle into the DoubleRow [Ki=32, 2] layout
                for n2 in range(TT // 1024):
                    n2sl = slice(n2 * 1024, (n2 + 1) * 1024)
                    for src, dst in ((qT_sb, qT8_sb), (kT_sb, kT8_sb)):
                        for h in range(2):
                            for i in range(2):
                                r0 = 64 * h + 32 * i
                                nc.sync.dma_start(
                                    out=dst[32 * h:32 * h + 32, i, n2sl],
                                    in_=src[r0:r0 + 32, n2sl])

            # ---- phase 3: attention, transposed orientation ----
            def emit_proj(tt, split=False):
                ob = outs.tile([P, D], BF16, tag="ob")
                for half in range(2):
                    po = psB.tile([P, 512], F32, tag="pv", name="po")
                    wsl = wproj_sb[:, half * 512:(half + 1) * 512]
                    if split:  # per-head halves: h0 part starts pre-epilogue(h1)
                        nc.tensor.matmul(
                            po[:], yT_sb[0:DH, tt * P:(tt + 1) * P], wsl[0:DH, :],
                            start=True, stop=False)
                        nc.tensor.matmul(
                            po[:], yT_sb[DH:P, tt * P:(tt + 1) * P], wsl[DH:P, :],
                            start=False, stop=True)
                    else:
                        nc.tensor.matmul(
                            po[:], yT_sb[:, tt * P:(tt + 1) * P], wsl,
                            start=True, stop=True)
                    nc.vector.tensor_copy(out=ob[:, half * 512:(half + 1) * 512],
                                          in_=po[:])
                nc.sync.dma_start(out=out[tt * P:(tt + 1) * P, :], in_=ob)

            # proj of span s is deferred into span s+1's k-tile loop so the
            # pv-slot rotation never blocks the next span's first PV matmul
            pending = []
            jtick = 0
            for b in range(B):
                for s in range(NSP):
                    qg = b * T + s * SPAN          # global q col base
                    njs = 4 * s + 4                # k-tiles for this span
                    pvs = [psB.tile([DH + 1, SPAN], F32, tag="pv", name=f"pv{_h}")
                           for _h in range(HPC)]
                    for j in range(njs):
                        off = max(0, j - 4 * s) * P
                        kb = b * T + j * P
                        st2 = psA.tile([P, 2, 512], F32, tag="b1", name="st2")
                        pt2 = ptp.tile([P, 2, 512], BF16, tag="pt", name="pt2")
                        for h in range(HPC):
                            if FP8_S:
                                nc.tensor.matmul(
                                    st2[:, h, off:SPAN],
                                    kT8_sb[32 * h:32 * h + 32, :, kb:kb + P],
                                    qT8_sb[32 * h:32 * h + 32, :,
                                           qg + off:qg + SPAN],
                                    start=True, stop=True,
                                    perf_mode=mybir.MatmulPerfMode.DoubleRow,
                                )
                            else:
                                hb = h * DH
                                nc.tensor.matmul(
                                    st2[:, h, off:SPAN],
                                    kT_sb[hb:hb + DH, kb:kb + P],
                                    qT_sb[hb:hb + DH, qg + off:qg + SPAN],
                                    start=True, stop=True,
                                )
                        nc.scalar.activation(
                            pt2[:, :, off:SPAN], st2[:, :, off:SPAN],
                            AF.Exp, bias=mbias_sb[:, b, j:j + 1])
                        for h in range(HPC):
                            if j >= 4 * s:  # diagonal block: keep q >= k
                                nc.vector.tensor_tensor(
                                    pt2[:, h, off:off + P], pt2[:, h, off:off + P],
                                    ut_sb[:], OP.mult)
                            vc0 = h * (DH + 1)
                            nc.tensor.matmul(
                                pvs[h][:, off:SPAN],
                                v_nat[:, b * NKT + j, vc0:vc0 + DH + 1],
                                pt2[:, h, off:SPAN],
                                start=(j == 0), stop=(j == njs - 1),
                            )
                        if pending:
                            emit_proj(pending.pop(0))
                        jtick += 1
                        if qkv_todo and jtick % 3 == 0:
                            emit_qkv(*qkv_todo.pop(0))
                    for h in range(HPC):
                        den = rows.tile([1, SPAN], F32, tag="den")
                        nc.vector.tensor_tensor(
                            den, pvs[h][DH:DH + 1, :],
                            mrowinv_sb[0:1, qg:qg + SPAN], OP.add)
                        rq = rows.tile([1, SPAN], F32, tag="rq")
                        nc.vector.reciprocal_approx_fast(out=rq, in_=den)
                        bc_sb = rows.tile([DH, SPAN], F32, tag="bcs")
                        nc.gpsimd.partition_broadcast(bc_sb[:], rq[:])
                        hb = h * DH
                        nc.vector.tensor_tensor(
                            yT_sb[hb:hb + DH, qg:qg + SPAN],
                            pvs[h][0:DH, :], bc_sb[:], OP.mult)
                    pending.extend(range(qg // P, (qg + SPAN) // P))
            for tt in pending:
                emit_proj(tt)

    nc.finalize()
    return nc


@functools.lru_cache(maxsize=1)
def _built():
    return build()


def _prep_core(c, x, attention_mask, W_attn, b_attn, W_proj):
    bf = ml_dtypes.bfloat16
    q0 = c * HPC * DH
    qs = slice(q0, q0 + P)
    ks = slice(D + q0, D + q0 + P)
    vs = slice(2 * D + q0, 2 * D + q0 + P)
    wsl = np.concatenate(
        [W_attn[:, qs], W_attn[:, ks], W_attn[:, vs]], axis=1)  # [1024, 384]
    bq = b_attn[qs] * QSCALE
    if FP8_QKV:
        # [P, K2, 2, 3P]: DoubleRow pairs two 128-row k-tiles per matmul
        wq = wsl.reshape(K2, 2, P, 3 * P).transpose(2, 0, 1, 3)
        wq = np.ascontiguousarray(wq).astype(ml_dtypes.float8_e4m3)
    else:
        # [P, KC, 3P]: partition-major so the DMA is contiguous per partition
        wq = wsl.reshape(KC, P, 3 * P).transpose(1, 0, 2)
        wq = np.ascontiguousarray(wq).astype(bf)
    return {
        "wqkv": wq,
        "bqkv": np.ascontiguousarray(
            np.stack([bq, b_attn[ks], b_attn[vs]], axis=1)).astype(np.float32),
        "wproj": np.ascontiguousarray(W_proj[qs, :]).astype(bf),
    }


def build_in_maps(x, attention_mask, W_attn, b_attn, W_proj):
    bf = ml_dtypes.bfloat16
    x = np.asarray(x, dtype=np.float32)
    attention_mask = np.asarray(attention_mask)
    W_attn = np.asarray(W_attn, dtype=np.float32)
    b_attn = np.asarray(b_attn, dtype=np.float32)
    W_proj = np.asarray(W_proj, dtype=np.float32)

    xr = x.reshape(TT, D).T  # [D, TT]
    if FP8_QKV:
        xT = np.ascontiguousarray(
            xr.reshape(K2, 2, P, TT).transpose(2, 0, 1, 3)
        ).astype(ml_dtypes.float8_e4m3)  # [P, K2, 2, TT]
    else:
        xT = np.ascontiguousarray(xr).astype(bf)
    maskf = attention_mask.astype(np.float32)
    mrowinv = np.ascontiguousarray(
        ((1.0 - maskf) * 1e30 + 1e-20).reshape(1, TT)).astype(np.float32)
    # per-key exp bias: ESHIFT, plus MASKP for masked keys (exp -> ~0)
    mb = ESHIFT + MASKP * (1.0 - maskf)
    mbias = np.ascontiguousarray(
        mb.reshape(B, NKT, P).transpose(2, 0, 1)).astype(np.float32)  # [P,B,NKT]

    in_maps = []
    for c in range(NCORES):
        m = _prep_core(c, x, attention_mask, W_attn, b_attn, W_proj)
        m["xT"] = xT
        m["mrowinv"] = mrowinv
        m["mbias"] = mbias
        in_maps.append(m)
    return in_maps


def kernel(x, attention_mask, W_attn, b_attn, W_proj, b_proj):
    b_proj = np.asarray(b_proj, dtype=np.float32)
    nc = _built()
    in_maps = build_in_maps(x, attention_mask, W_attn, b_attn, W_proj)
    res = run_bass_kernel_spmd(nc, in_maps, core_ids=list(range(NCORES)))
    acc = np.zeros((TT, D), dtype=np.float32)
    for c in range(NCORES):
        acc += res.results[c]["out"].astype(np.float32)
    acc += b_proj[None, :]
    return acc.reshape(B, T, D)


# revision 36
# speedup vs baseline: 1.0482x; 1.0482x over previous
"""Trainium2 Bass kernel for CausalSelfAttention (B=2, T=2048, D=1024, H=16).

Sharding (8 cores): Megatron-style tensor parallel. Core c owns heads
{2c, 2c+1}: column-parallel c_attn (384 of 3072 output features),
full attention for its 2 heads x 2 batches, row-parallel c_proj
(128 of 1024 contraction rows). Host sums the 8 partial outputs and
adds b_proj.

Device algorithm (per core), all matmuls bf16, softmax f32:
  1. qkv^T = Wslice^T @ x^T   -- x^T arrives pre-transposed bf16 from host.
     q^T, k^T stay in SBUF; v^T tiles are PE-transposed (identity matmul)
     into natural [k, d] layout with a constant ones column appended
     (softmax denominator rides the PV matmul as row 64).
  2. Attention in the TRANSPOSED orientation, one 128-key tile at a time:
     S^T[k, q] = k^T.T @ q^T into the two halves of a [128, 2, 512] f32
     PSUM tile (2 banks); the two heads' S matmuls run CONCURRENTLY in
     the PE array (row groups 0-63 / 64-127 via auto tile_position) and
     write different banks. ONE exp per k-tile on ACT covers both heads
     straight out of PSUM, with a per-partition bias column that carries
     both the -10 shift (cancels in the softmax ratio; logits are O(1),
     so no max-subtraction) and the additive key-mask (-50 for masked
     keys, exp -> 0; the bias depends only on the k-tile so both heads
     share it). Causal: skip invalid column ranges + an upper-triangular
     multiplicative mask on diagonal blocks (DVE). The 2-slot two-bank
     PSUM rotation (tag shared with the padded qkv-phase accumulators)
     lets S of tile j+1 overlap exp of tile j.
  3. PV: out[65, q] = [v_h | ones].T @ P^T accumulated over k-tiles.
     Row 64 is the denominator (masked keys contribute exp(-60)~0).
     rq = qmask / (denom + eps) broadcast across partitions on GpSimd,
     multiplied into y^T on DVE.
  4. out = y^T.T @ Wproj_rows -> f32 PSUM, DVE cast to bf16 SBUF, DMA.
     Host sums the 8 partials + b_proj.
"""

import functools

import numpy as np
import ml_dtypes

import concourse.bass as bass
import concourse.mybir as mybir
import concourse.tile as tile
from concourse import bacc
from concourse.bass_utils import run_bass_kernel_spmd
from concourse.masks import make_upper_triangular, make_identity

BF16 = mybir.dt.bfloat16
F32 = mybir.dt.float32
AF = mybir.ActivationFunctionType
OP = mybir.AluOpType

B, T, D, NH = 2, 2048, 1024, 16
DH = 64                  # head dim
HPC = 2                  # heads per core
NCORES = 8
TT = B * T               # 4096 total tokens
P = 128
KC = D // P              # 8 contraction tiles for qkv
SPAN = 512               # q-span processed per softmax pass
NSP = T // SPAN          # 4 spans per batch
NKT = T // P             # 16 k-tiles per batch
QSCALE = 1.0 / np.sqrt(DH)
ESHIFT = -10.0           # constant exp shift; cancels in softmax ratio
MASKP = -50.0            # additive key-mask penalty (pre-exp)
VW = 2 * DH + 2          # v_nat width: [v_h0 | 1 | v_h1 | 1]
FP8_QKV = False          # fp8e4m3 DoubleRow qkv: fails rel-err gate (~4e-2)
FP8_S = False            # fp8 DoubleRow S: no gain (stream-bound, K=64 fits
                         # one matmul already) and costs accuracy (1.5e-2)
K2 = KC // 2             # DoubleRow k-tile pairs
FP8 = mybir.dt.float8e4


def build():
    nc = bacc.Bacc(None)

    if FP8_QKV:
        xT = nc.dram_tensor("xT", [P, K2, 2, TT], FP8, kind="ExternalInput")
        wqkv = nc.dram_tensor("wqkv", [P, K2, 2, 3 * P], FP8,
                              kind="ExternalInput")
    else:
        xT = nc.dram_tensor("xT", [D, TT], BF16, kind="ExternalInput")
        wqkv = nc.dram_tensor("wqkv", [P, KC, 3 * P], BF16, kind="ExternalInput")
    bqkv = nc.dram_tensor("bqkv", [P, 3], F32, kind="ExternalInput")
    wproj = nc.dram_tensor("wproj", [P, D], BF16, kind="ExternalInput")
    mrowinv = nc.dram_tensor("mrowinv", [1, TT], F32, kind="ExternalInput")
    mbias = nc.dram_tensor("mbias", [P, B, NKT], F32, kind="ExternalInput")
    out = nc.dram_tensor("out", [TT, D], BF16, kind="ExternalOutput")

    with tile.TileContext(nc) as tc:
        with (
            tc.tile_pool(name="singles", bufs=1) as singles,
            tc.tile_pool(name="stage", bufs=3) as stage,
            tc.tile_pool(name="pt", bufs=6) as ptp,
            tc.tile_pool(name="rows", bufs=4) as rows,
            tc.tile_pool(name="outs", bufs=4) as outs,
            # 2 two-bank slots shared by qkv accumulators and paired S^T tiles
            tc.tile_pool(name="psA", bufs=2, space="PSUM") as psA,
            # 4 one-bank slots shared by pv accumulators, proj out, transposes
            tc.tile_pool(name="psB", bufs=4, space="PSUM") as psB,
        ):
            # ---- constants / weights (small DMAs first: casts block on them) ----
            if FP8_QKV:
                wqkv_sb = singles.tile([P, K2, 2, 3 * P], FP8)
                nc.sync.dma_start(out=wqkv_sb, in_=wqkv[:, :, :, :])
            else:
                wqkv_sb = singles.tile([P, KC, 3 * P], BF16)
                nc.sync.dma_start(out=wqkv_sb, in_=wqkv[:, :, :])
            bqkv_sb = singles.tile([P, 3], F32)
            nc.sync.dma_start(out=bqkv_sb, in_=bqkv[:, :])
            if FP8_QKV:
                xT_sb = singles.tile([P, K2, 2, TT], FP8)

                def dma_x(k, tsl):
                    nc.sync.dma_start(out=xT_sb[:, k, :, tsl],
                                      in_=xT[:, k, :, tsl])
                nkx = K2
            else:
                xT_sb = singles.tile([P, KC, TT], BF16)

                def dma_x(k, tsl):
                    nc.sync.dma_start(out=xT_sb[:, k, tsl],
                                      in_=xT[k * P:(k + 1) * P, tsl])
                nkx = KC
            # first qkv group only needs [all k, n2=0]: fetch those before the
            # small attention-phase constants so the PE starts earlier
            for k in range(nkx):
                dma_x(k, slice(0, 1024))
            wproj_sb = singles.tile([P, D], BF16)
            nc.sync.dma_start(out=wproj_sb, in_=wproj[:, :])
            mrowinv_sb = singles.tile([1, TT], F32)
            nc.sync.dma_start(out=mrowinv_sb, in_=mrowinv[:, :])
            mbias_sb = singles.tile([P, B, NKT], F32)
            nc.sync.dma_start(out=mbias_sb, in_=mbias[:, :, :])
            for n2 in range(1, TT // 1024):
                for k in range(nkx):
                    dma_x(k, slice(n2 * 1024, (n2 + 1) * 1024))

            ut_sb = singles.tile([P, P], BF16)  # keep q >= k
            make_upper_triangular(nc, ut_sb, val=1.0, diag=True)
            ident = singles.tile([P, P], BF16)
            make_identity(nc, ident)

            QKDT = FP8 if FP8_S else BF16
            qT_sb = singles.tile([P, TT], QKDT)   # rows: h0 d0..63 | h1 d0..63
            kT_sb = singles.tile([P, TT], QKDT)
            if FP8_S:
                # DoubleRow layout: [32h + d%32, d//32, t]
                qT8_sb = singles.tile([64, 2, TT], FP8)
                kT8_sb = singles.tile([64, 2, TT], FP8)
            yT_sb = singles.tile([P, TT], BF16)
            v_nat = singles.tile([P, NKT * B, VW], BF16)
            # denominator ones columns (64 and 129), constant across tiles
            nc.vector.memset(v_nat[:, :, DH:DH + 1], 1.0)
            nc.vector.memset(v_nat[:, :, VW - 1:VW], 1.0)

            # ---- phase 1: qkv^T = W^T @ x^T ----
            # [128 feat, 512 t] accumulators; n2-outer so attention starts
            # early. Accumulators alternate between the psA slots and the
            # psB slots (idle until attention) for a deeper group pipeline.
            gidx = 0
            for n2 in range(TT // 1024):
                for m in range(3):
                    for h2 in range(2):
                        if gidx % 2 == 0:
                            pq = psA.tile([P, 512], F32, tag="b1", name="pq",
                                          padded_shape=[P, 1024])
                        else:
                            pq = psB.tile([P, 512], F32, tag="pv", name="pq")
                        gidx += 1
                        t0 = n2 * 1024 + h2 * 512
                        if FP8_QKV:
                            for k2 in range(K2):
                                nc.tensor.matmul(
                                    pq[:],
                                    wqkv_sb[:, k2, :, m * P:(m + 1) * P],
                                    xT_sb[:, k2, :, t0:t0 + 512],
                                    start=(k2 == 0), stop=(k2 == K2 - 1),
                                    perf_mode=mybir.MatmulPerfMode.DoubleRow,
                                )
                        else:
                            for k in range(KC):
                                nc.tensor.matmul(
                                    pq[:],
                                    wqkv_sb[:, k, m * P:(m + 1) * P],
                                    xT_sb[:, k, t0:t0 + 512],
                                    start=(k == 0), stop=(k == KC - 1),
                                )
                        tcols = slice(t0, t0 + 512)
                        if m == 0:
                            nc.scalar.activation(
                                qT_sb[:, tcols], pq[:], AF.Identity,
                                bias=bqkv_sb[:, 0:1], scale=QSCALE)
                        elif m == 1:
                            nc.scalar.activation(
                                kT_sb[:, tcols], pq[:], AF.Identity,
                                bias=bqkv_sb[:, 1:2], scale=1.0)
                        else:
                            vst = stage.tile([P, 512], BF16, tag="vst")
                            nc.scalar.activation(
                                vst[:], pq[:], AF.Identity,
                                bias=bqkv_sb[:, 2:3], scale=1.0)
                            # phase 2: v natural [k, d] via PE transpose
                            for jj in range(512 // P):
                                j32 = n2 * 8 + h2 * 4 + jj
                                vtp = psB.tile([P, P], BF16, tag="pv")
                                nc.tensor.transpose(
                                    vtp[:], vst[:, jj * P:(jj + 1) * P], ident[:])
                                nc.vector.tensor_copy(
                                    out=v_nat[:, j32, 0:DH], in_=vtp[:, 0:DH])
                                nc.vector.tensor_copy(
                                    out=v_nat[:, j32, DH + 1:2 * DH + 1],
                                    in_=vtp[:, DH:2 * DH])
                if FP8_S:
                    # partition shuffle into the DoubleRow [Ki=32, 2] layout
                    n2sl = slice(n2 * 1024, (n2 + 1) * 1024)
                    for src, dst in ((qT_sb, qT8_sb), (kT_sb, kT8_sb)):
                        for h in range(2):
                            for i in range(2):
                                r0 = 64 * h + 32 * i
                                nc.sync.dma_start(
                                    out=dst[32 * h:32 * h + 32, i, n2sl],
                                    in_=src[r0:r0 + 32, n2sl])

            # ---- phase 3: attention, transposed orientation ----
            def emit_proj(tt, split=False):
                ob = outs.tile([P, D], BF16, tag="ob")
                for half in range(2):
                    po = psB.tile([P, 512], F32, tag="pv", name="po")
                    wsl = wproj_sb[:, half * 512:(half + 1) * 512]
                    if split:  # per-head halves: h0 part starts pre-epilogue(h1)
                        nc.tensor.matmul(
                            po[:], yT_sb[0:DH, tt * P:(tt + 1) * P], wsl[0:DH, :],
                            start=True, stop=False)
                        nc.tensor.matmul(
                            po[:], yT_sb[DH:P, tt * P:(tt + 1) * P], wsl[DH:P, :],
                            start=False, stop=True)
                    else:
                        nc.tensor.matmul(
                            po[:], yT_sb[:, tt * P:(tt + 1) * P], wsl,
                            start=True, stop=True)
                    nc.vector.tensor_copy(out=ob[:, half * 512:(half + 1) * 512],
                                          in_=po[:])
                nc.sync.dma_start(out=out[tt * P:(tt + 1) * P, :], in_=ob)

            # proj of span s is deferred into span s+1's k-tile loop so the
            # pv-slot rotation never blocks the next span's first PV matmul
            pending = []
            for b in range(B):
                for s in range(NSP):
                    qg = b * T + s * SPAN          # global q col base
                    njs = 4 * s + 4                # k-tiles for this span
                    pvs = [psB.tile([DH + 1, SPAN], F32, tag="pv", name=f"pv{_h}")
                           for _h in range(HPC)]
                    stride = max(1, njs // 4)  # spread deferred proj evenly
                    for j in range(njs):
                        off = max(0, j - 4 * s) * P
                        kb = b * T + j * P
                        st2 = psA.tile([P, 2, 512], F32, tag="b1", name="st2")
                        pt2 = ptp.tile([P, 2, 512], BF16, tag="pt", name="pt2")
                        for h in range(HPC):
                            if FP8_S:
                                nc.tensor.matmul(
                                    st2[:, h, off:SPAN],
                                    kT8_sb[32 * h:32 * h + 32, :, kb:kb + P],
                                    qT8_sb[32 * h:32 * h + 32, :,
                                           qg + off:qg + SPAN],
                                    start=True, stop=True,
                                    perf_mode=mybir.MatmulPerfMode.DoubleRow,
                                )
                            else:
                                hb = h * DH
                                nc.tensor.matmul(
                                    st2[:, h, off:SPAN],
                                    kT_sb[hb:hb + DH, kb:kb + P],
                                    qT_sb[hb:hb + DH, qg + off:qg + SPAN],
                                    start=True, stop=True,
                                )
                        nc.scalar.activation(
                            pt2[:, :, off:SPAN], st2[:, :, off:SPAN],
                            AF.Exp, bias=mbias_sb[:, b, j:j + 1])
                        for h in range(HPC):
                            if j >= 4 * s:  # diagonal block: keep q >= k
                                nc.vector.tensor_tensor(
                                    pt2[:, h, off:off + P], pt2[:, h, off:off + P],
                                    ut_sb[:], OP.mult)
                            vc0 = h * (DH + 1)
                            nc.tensor.matmul(
                                pvs[h][:, off:SPAN],
                                v_nat[:, b * NKT + j, vc0:vc0 + DH + 1],
                                pt2[:, h, off:SPAN],
                                start=(j == 0), stop=(j == njs - 1),
                            )
                        if pending and j % stride == stride - 1:
                            emit_proj(pending.pop(0))
                    for h in range(HPC):
                        den = rows.tile([1, SPAN], F32, tag="den")
                        nc.vector.tensor_tensor(
                            den, pvs[h][DH:DH + 1, :],
                            mrowinv_sb[0:1, qg:qg + SPAN], OP.add)
                        rq = rows.tile([1, SPAN], F32, tag="rq")
                        nc.vector.reciprocal_approx_fast(out=rq, in_=den)
                        bc_sb = rows.tile([DH, SPAN], F32, tag="bcs")
                        nc.gpsimd.partition_broadcast(bc_sb[:], rq[:])
                        hb = h * DH
                        nc.vector.tensor_tensor(
                            yT_sb[hb:hb + DH, qg:qg + SPAN],
                            pvs[h][0:DH, :], bc_sb[:], OP.mult)
                    pending.extend(range(qg // P, (qg + SPAN) // P))
            for tt in pending:
                emit_proj(tt)

    nc.finalize()
    return nc


@functools.lru_cache(maxsize=1)
def _built():
    return build()


def _prep_core(c, x, attention_mask, W_attn, b_attn, W_proj):
    bf = ml_dtypes.bfloat16
    q0 = c * HPC * DH
    qs = slice(q0, q0 + P)
    ks = slice(D + q0, D + q0 + P)
    vs = slice(2 * D + q0, 2 * D + q0 + P)
    wsl = np.concatenate(
        [W_attn[:, qs], W_attn[:, ks], W_attn[:, vs]], axis=1)  # [1024, 384]
    bq = b_attn[qs] * QSCALE
    if FP8_QKV:
        # [P, K2, 2, 3P]: DoubleRow pairs two 128-row k-tiles per matmul
        wq = wsl.reshape(K2, 2, P, 3 * P).transpose(2, 0, 1, 3)
        wq = np.ascontiguousarray(wq).astype(ml_dtypes.float8_e4m3)
    else:
        # [P, KC, 3P]: partition-major so the DMA is contiguous per partition
        wq = wsl.reshape(KC, P, 3 * P).transpose(1, 0, 2)
        wq = np.ascontiguousarray(wq).astype(bf)
    return {
        "wqkv": wq,
        "bqkv": np.ascontiguousarray(
            np.stack([bq, b_attn[ks], b_attn[vs]], axis=1)).astype(np.float32),
        "wproj": np.ascontiguousarray(W_proj[qs, :]).astype(bf),
    }


def build_in_maps(x, attention_mask, W_attn, b_attn, W_proj):
    bf = ml_dtypes.bfloat16
    x = np.asarray(x, dtype=np.float32)
    attention_mask = np.asarray(attention_mask)
    W_attn = np.asarray(W_attn, dtype=np.float32)
    b_attn = np.asarray(b_attn, dtype=np.float32)
    W_proj = np.asarray(W_proj, dtype=np.float32)

    xr = x.reshape(TT, D).T  # [D, TT]
    if FP8_QKV:
        xT = np.ascontiguousarray(
            xr.reshape(K2, 2, P, TT).transpose(2, 0, 1, 3)
        ).astype(ml_dtypes.float8_e4m3)  # [P, K2, 2, TT]
    else:
        xT = np.ascontiguousarray(xr).astype(bf)
    maskf = attention_mask.astype(np.float32)
    mrowinv = np.ascontiguousarray(
        ((1.0 - maskf) * 1e30 + 1e-20).reshape(1, TT)).astype(np.float32)
    # per-key exp bias: ESHIFT, plus MASKP for masked keys (exp -> ~0)
    mb = ESHIFT + MASKP * (1.0 - maskf)
    mbias = np.ascontiguousarray(
        mb.reshape(B, NKT, P).transpose(2, 0, 1)).astype(np.float32)  # [P,B,NKT]

    in_maps = []
    for c in range(NCORES):
        m = _prep_core(c, x, attention_mask, W_attn, b_attn, W_proj)
        m["xT"] = xT
        m["mrowinv"] = mrowinv
        m["mbias"] = mbias
        in_maps.append(m)
    return in_maps


def kernel(x, attention_mask, W_attn, b_attn, W_proj, b_proj):
    b_proj = np.asarray(b_proj, dtype=np.float32)
    nc = _built()
    in_maps = build_in_maps(x, attention_mask, W_attn, b_attn, W_proj)
    res = run_bass_kernel_spmd(nc, in_maps, core_ids=list(range(NCORES)))
    acc = np.zeros((TT, D), dtype=np.float32)
    for c in range(NCORES):
        acc += res.results[c]["out"].astype(np.float32)
    acc += b_proj[None, :]
    return acc.reshape(B, T, D)


# revision 42
# speedup vs baseline: 1.0683x; 1.0192x over previous
"""Trainium2 Bass kernel for CausalSelfAttention (B=2, T=2048, D=1024, H=16).

Sharding (8 cores): Megatron-style tensor parallel. Core c owns heads
{2c, 2c+1}: column-parallel c_attn (384 of 3072 output features),
full attention for its 2 heads x 2 batches, row-parallel c_proj
(128 of 1024 contraction rows). Host sums the 8 partial outputs and
adds b_proj.

Device algorithm (per core), all matmuls bf16, softmax f32:
  1. qkv^T = Wslice^T @ x^T   -- x^T arrives pre-transposed bf16 from host.
     q^T, k^T stay in SBUF; v^T tiles are PE-transposed (identity matmul)
     into natural [k, d] layout with a constant ones column appended
     (softmax denominator rides the PV matmul as row 64).
  2. Attention in the TRANSPOSED orientation, one 128-key tile at a time:
     S^T[k, q] = k^T.T @ q^T into the two halves of a [128, 2, 512] f32
     PSUM tile (2 banks); the two heads' S matmuls run CONCURRENTLY in
     the PE array (row groups 0-63 / 64-127 via auto tile_position) and
     write different banks. ONE exp per k-tile on ACT covers both heads
     straight out of PSUM, with a per-partition bias column that carries
     both the -10 shift (cancels in the softmax ratio; logits are O(1),
     so no max-subtraction) and the additive key-mask (-50 for masked
     keys, exp -> 0; the bias depends only on the k-tile so both heads
     share it). Causal: skip invalid column ranges + an upper-triangular
     multiplicative mask on diagonal blocks (DVE). The 2-slot two-bank
     PSUM rotation (tag shared with the padded qkv-phase accumulators)
     lets S of tile j+1 overlap exp of tile j.
  3. PV: out[65, q] = [v_h | ones].T @ P^T accumulated over k-tiles.
     Row 64 is the denominator (masked keys contribute exp(-60)~0).
     rq = qmask / (denom + eps) broadcast across partitions on GpSimd,
     multiplied into y^T on DVE.
  4. out = y^T.T @ Wproj_rows -> f32 PSUM, DVE cast to bf16 SBUF, DMA.
     Host sums the 8 partials + b_proj.
"""

import functools

import numpy as np
import ml_dtypes

import concourse.bass as bass
import concourse.mybir as mybir
import concourse.tile as tile
from concourse import bacc
from concourse.bass_utils import run_bass_kernel_spmd
from concourse.masks import make_upper_triangular, make_identity

BF16 = mybir.dt.bfloat16
F32 = mybir.dt.float32
AF = mybir.ActivationFunctionType
OP = mybir.AluOpType

B, T, D, NH = 2, 2048, 1024, 16
DH = 64                  # head dim
HPC = 2                  # heads per core
NCORES = 8
TT = B * T               # 4096 total tokens
P = 128
KC = D // P              # 8 contraction tiles for qkv
SPAN = 512               # q-span processed per softmax pass
NSP = T // SPAN          # 4 spans per batch
NKT = T // P             # 16 k-tiles per batch
QSCALE = 1.0 / np.sqrt(DH)
ESHIFT = -10.0           # constant exp shift; cancels in softmax ratio
MASKP = -50.0            # additive key-mask penalty (pre-exp)
VW = 2 * DH + 2          # v_nat width: [v_h0 | 1 | v_h1 | 1]
FP8_QKV = False          # fp8e4m3 DoubleRow qkv: fails rel-err gate (~4e-2)
FP8_S = False            # fp8 DoubleRow S: no gain (stream-bound, K=64 fits
                         # one matmul already) and costs accuracy (1.5e-2)
K2 = KC // 2             # DoubleRow k-tile pairs
FP8 = mybir.dt.float8e4


def build():
    nc = bacc.Bacc(None)

    if FP8_QKV:
        xT = nc.dram_tensor("xT", [P, K2, 2, TT], FP8, kind="ExternalInput")
        wqkv = nc.dram_tensor("wqkv", [P, K2, 2, 3 * P], FP8,
                              kind="ExternalInput")
    else:
        xT = nc.dram_tensor("xT", [D, TT], BF16, kind="ExternalInput")
        wqkv = nc.dram_tensor("wqkv", [P, KC, 3 * P], BF16, kind="ExternalInput")
    bqkv = nc.dram_tensor("bqkv", [P, 3], F32, kind="ExternalInput")
    wproj = nc.dram_tensor("wproj", [P, D], BF16, kind="ExternalInput")
    mrowinv = nc.dram_tensor("mrowinv", [1, TT], F32, kind="ExternalInput")
    mbias = nc.dram_tensor("mbias", [P, B, NKT], F32, kind="ExternalInput")
    out = nc.dram_tensor("out", [TT, D], BF16, kind="ExternalOutput")

    with tile.TileContext(nc) as tc:
        with (
            tc.tile_pool(name="singles", bufs=1) as singles,
            tc.tile_pool(name="stage", bufs=3) as stage,
            tc.tile_pool(name="pt", bufs=6) as ptp,
            tc.tile_pool(name="rows", bufs=4) as rows,
            tc.tile_pool(name="outs", bufs=4) as outs,
            # 2 two-bank slots shared by qkv accumulators and paired S^T tiles
            tc.tile_pool(name="psA", bufs=2, space="PSUM") as psA,
            # 4 one-bank slots shared by pv accumulators, proj out, transposes
            tc.tile_pool(name="psB", bufs=4, space="PSUM") as psB,
        ):
            # ---- constants / weights (small DMAs first: casts block on them) ----
            if FP8_QKV:
                wqkv_sb = singles.tile([P, K2, 2, 3 * P], FP8)
                nc.sync.dma_start(out=wqkv_sb, in_=wqkv[:, :, :, :])
            else:
                wqkv_sb = singles.tile([P, KC, 3 * P], BF16)
                nc.sync.dma_start(out=wqkv_sb, in_=wqkv[:, :, :])
            bqkv_sb = singles.tile([P, 3], F32)
            nc.sync.dma_start(out=bqkv_sb, in_=bqkv[:, :])
            if FP8_QKV:
                xT_sb = singles.tile([P, K2, 2, TT], FP8)

                def dma_x(k, tsl):
                    nc.sync.dma_start(out=xT_sb[:, k, :, tsl],
                                      in_=xT[:, k, :, tsl])
                nkx = K2
            else:
                xT_sb = singles.tile([P, KC, TT], BF16)

                def dma_x(k, tsl):
                    nc.sync.dma_start(out=xT_sb[:, k, tsl],
                                      in_=xT[k * P:(k + 1) * P, tsl])
                nkx = KC
            for n2 in range(TT // 1024):
                for k in range(nkx):
                    dma_x(k, slice(n2 * 1024, (n2 + 1) * 1024))

            ut_sb = singles.tile([P, P], BF16)  # keep q >= k
            make_upper_triangular(nc, ut_sb, val=1.0, diag=True)
            ident = singles.tile([P, P], BF16)
            make_identity(nc, ident)

            QKDT = FP8 if FP8_S else BF16
            qT_sb = singles.tile([P, TT], QKDT)   # rows: h0 d0..63 | h1 d0..63
            kT_sb = singles.tile([P, TT], QKDT)
            if FP8_S:
                # DoubleRow layout: [32h + d%32, d//32, t]
                qT8_sb = singles.tile([64, 2, TT], FP8)
                kT8_sb = singles.tile([64, 2, TT], FP8)
            yT_sb = singles.tile([P, TT], BF16)
            v_nat = singles.tile([P, NKT * B, VW], BF16)
            # denominator ones columns (64 and 129), constant across tiles
            nc.vector.memset(v_nat[:, :, DH:DH + 1], 1.0)
            nc.vector.memset(v_nat[:, :, VW - 1:VW], 1.0)

            # ---- phase 1: qkv^T = W^T @ x^T ----
            # [128 feat, 512 t] accumulators; n2-outer so attention starts
            # early. Accumulators alternate between the psA slots and the
            # psB slots (idle until attention) for a deeper group pipeline.
            gidx = 0
            for n2 in range(TT // 1024):
                for m in range(3):
                    for h2 in range(2):
                        if gidx % 2 == 0:
                            pq = psA.tile([P, 512], F32, tag="b1", name="pq",
                                          padded_shape=[P, 1024])
                        else:
                            pq = psB.tile([P, 512], F32, tag="pv", name="pq")
                        gidx += 1
                        t0 = n2 * 1024 + h2 * 512
                        if FP8_QKV:
                            for k2 in range(K2):
                                nc.tensor.matmul(
                                    pq[:],
                                    wqkv_sb[:, k2, :, m * P:(m + 1) * P],
                                    xT_sb[:, k2, :, t0:t0 + 512],
                                    start=(k2 == 0), stop=(k2 == K2 - 1),
                                    perf_mode=mybir.MatmulPerfMode.DoubleRow,
                                )
                        else:
                            for k in range(KC):
                                nc.tensor.matmul(
                                    pq[:],
                                    wqkv_sb[:, k, m * P:(m + 1) * P],
                                    xT_sb[:, k, t0:t0 + 512],
                                    start=(k == 0), stop=(k == KC - 1),
                                )
                        tcols = slice(t0, t0 + 512)
                        if m == 0:
                            nc.scalar.activation(
                                qT_sb[:, tcols], pq[:], AF.Identity,
                                bias=bqkv_sb[:, 0:1], scale=QSCALE)
                        elif m == 1:
                            nc.scalar.activation(
                                kT_sb[:, tcols], pq[:], AF.Identity,
                                bias=bqkv_sb[:, 1:2], scale=1.0)
                        else:
                            vst = stage.tile([P, 512], BF16, tag="vst")
                            nc.scalar.activation(
                                vst[:], pq[:], AF.Identity,
                                bias=bqkv_sb[:, 2:3], scale=1.0)
                            # phase 2: v natural [k, d] via PE transpose
                            for jj in range(512 // P):
                                j32 = n2 * 8 + h2 * 4 + jj
                                vtp = psB.tile([P, P], BF16, tag="pv")
                                nc.tensor.transpose(
                                    vtp[:], vst[:, jj * P:(jj + 1) * P], ident[:])
                                nc.vector.tensor_copy(
                                    out=v_nat[:, j32, 0:DH], in_=vtp[:, 0:DH])
                                nc.vector.tensor_copy(
                                    out=v_nat[:, j32, DH + 1:2 * DH + 1],
                                    in_=vtp[:, DH:2 * DH])
                if FP8_S:
                    # partition shuffle into the DoubleRow [Ki=32, 2] layout
                    n2sl = slice(n2 * 1024, (n2 + 1) * 1024)
                    for src, dst in ((qT_sb, qT8_sb), (kT_sb, kT8_sb)):
                        for h in range(2):
                            for i in range(2):
                                r0 = 64 * h + 32 * i
                                nc.sync.dma_start(
                                    out=dst[32 * h:32 * h + 32, i, n2sl],
                                    in_=src[r0:r0 + 32, n2sl])

            # attention-phase constants: issued after the qkv groups so the
            # first matmul's DMA semaphore target stays minimal
            wproj_sb = singles.tile([P, D], BF16)
            nc.sync.dma_start(out=wproj_sb, in_=wproj[:, :])
            mrowinv_sb = singles.tile([1, TT], F32)
            nc.sync.dma_start(out=mrowinv_sb, in_=mrowinv[:, :])
            mbias_sb = singles.tile([P, B, NKT], F32)
            nc.sync.dma_start(out=mbias_sb, in_=mbias[:, :, :])

            # ---- phase 3: attention, transposed orientation ----
            def emit_proj(tt, tail=False):
                ob = outs.tile([P, D], BF16, tag="ob")
                for half in range(2):
                    po = psB.tile([P, 512], F32, tag="pv", name="po")
                    wsl = wproj_sb[:, half * 512:(half + 1) * 512]
                    nc.tensor.matmul(
                        po[:], yT_sb[:, tt * P:(tt + 1) * P], wsl,
                        start=True, stop=True)
                    osl = slice(half * 512, (half + 1) * 512)
                    if tail and half == 1:  # drain on two engines at the end
                        nc.scalar.activation(ob[:, osl], po[:], AF.Identity)
                    else:
                        nc.vector.tensor_copy(out=ob[:, osl], in_=po[:])
                nc.sync.dma_start(out=out[tt * P:(tt + 1) * P, :], in_=ob)

            # proj of span s is deferred into span s+1's k-tile loop so the
            # pv-slot rotation never blocks the next span's first PV matmul
            pending = []
            for b in range(B):
                for s in range(NSP):
                    qg = b * T + s * SPAN          # global q col base
                    njs = 4 * s + 4                # k-tiles for this span
                    pvs = [psB.tile([DH + 1, SPAN], F32, tag="pv", name=f"pv{_h}")
                           for _h in range(HPC)]
                    # deferred-proj emission slots: back half of the span, so
                    # the proj never heads the tensor queue while waiting on
                    # the previous span's epilogue (in-order queue stall)
                    if njs == 4:
                        slots = [2, 2, 3, 3]
                    else:
                        step = max(1, njs // 8)
                        slots = [njs - 1 - (3 - i) * step for i in range(4)]
                    si = 0
                    for j in range(njs):
                        off = max(0, j - 4 * s) * P
                        kb = b * T + j * P
                        st2 = psA.tile([P, 2, 512], F32, tag="b1", name="st2")
                        pt2 = ptp.tile([P, 2, 512], BF16, tag="pt", name="pt2")
                        for h in range(HPC):
                            if FP8_S:
                                nc.tensor.matmul(
                                    st2[:, h, off:SPAN],
                                    kT8_sb[32 * h:32 * h + 32, :, kb:kb + P],
                                    qT8_sb[32 * h:32 * h + 32, :,
                                           qg + off:qg + SPAN],
                                    start=True, stop=True,
                                    perf_mode=mybir.MatmulPerfMode.DoubleRow,
                                )
                            else:
                                hb = h * DH
                                nc.tensor.matmul(
                                    st2[:, h, off:SPAN],
                                    kT_sb[hb:hb + DH, kb:kb + P],
                                    qT_sb[hb:hb + DH, qg + off:qg + SPAN],
                                    start=True, stop=True,
                                )
                        nc.scalar.activation(
                            pt2[:, :, off:SPAN], st2[:, :, off:SPAN],
                            AF.Exp, bias=mbias_sb[:, b, j:j + 1])
                        for h in range(HPC):
                            if j >= 4 * s:  # diagonal block: keep q >= k
                                nc.vector.tensor_tensor(
                                    pt2[:, h, off:off + P], pt2[:, h, off:off + P],
                                    ut_sb[:], OP.mult)
                            vc0 = h * (DH + 1)
                            nc.tensor.matmul(
                                pvs[h][:, off:SPAN],
                                v_nat[:, b * NKT + j, vc0:vc0 + DH + 1],
                                pt2[:, h, off:SPAN],
                                start=(j == 0), stop=(j == njs - 1),
                            )
                        while pending and si < 4 and j == slots[si]:
                            emit_proj(pending.pop(0))
                            si += 1
                    for h in range(HPC):
                        den = rows.tile([1, SPAN], F32, tag="den")
                        nc.vector.tensor_tensor(
                            den, pvs[h][DH:DH + 1, :],
                            mrowinv_sb[0:1, qg:qg + SPAN], OP.add)
                        rq = rows.tile([1, SPAN], F32, tag="rq")
                        nc.vector.reciprocal_approx_fast(out=rq, in_=den)
                        bc_sb = rows.tile([DH, SPAN], F32, tag="bcs")
                        nc.gpsimd.partition_broadcast(bc_sb[:], rq[:])
                        hb = h * DH
                        nc.vector.tensor_tensor(
                            yT_sb[hb:hb + DH, qg:qg + SPAN],
                            pvs[h][0:DH, :], bc_sb[:], OP.mult)
                    pending.extend(range(qg // P, (qg + SPAN) // P))
            for tt in pending:
                emit_proj(tt, tail=True)

    nc.finalize()
    return nc


@functools.lru_cache(maxsize=1)
def _built():
    return build()


def _prep_core(c, x, attention_mask, W_attn, b_attn, W_proj):
    bf = ml_dtypes.bfloat16
    q0 = c * HPC * DH
    qs = slice(q0, q0 + P)
    ks = slice(D + q0, D + q0 + P)
    vs = slice(2 * D + q0, 2 * D + q0 + P)
    wsl = np.concatenate(
        [W_attn[:, qs], W_attn[:, ks], W_attn[:, vs]], axis=1)  # [1024, 384]
    bq = b_attn[qs] * QSCALE
    if FP8_QKV:
        # [P, K2, 2, 3P]: DoubleRow pairs two 128-row k-tiles per matmul
        wq = wsl.reshape(K2, 2, P, 3 * P).transpose(2, 0, 1, 3)
        wq = np.ascontiguousarray(wq).astype(ml_dtypes.float8_e4m3)
    else:
        # [P, KC, 3P]: partition-major so the DMA is contiguous per partition
        wq = wsl.reshape(KC, P, 3 * P).transpose(1, 0, 2)
        wq = np.ascontiguousarray(wq).astype(bf)
    return {
        "wqkv": wq,
        "bqkv": np.ascontiguousarray(
            np.stack([bq, b_attn[ks], b_attn[vs]], axis=1)).astype(np.float32),
        "wproj": np.ascontiguousarray(W_proj[qs, :]).astype(bf),
    }


def build_in_maps(x, attention_mask, W_attn, b_attn, W_proj):
    bf = ml_dtypes.bfloat16
    x = np.asarray(x, dtype=np.float32)
    attention_mask = np.asarray(attention_mask)
    W_attn = np.asarray(W_attn, dtype=np.float32)
    b_attn = np.asarray(b_attn, dtype=np.float32)
    W_proj = np.asarray(W_proj, dtype=np.float32)

    xr = x.reshape(TT, D).T  # [D, TT]
    if FP8_QKV:
        xT = np.ascontiguousarray(
            xr.reshape(K2, 2, P, TT).transpose(2, 0, 1, 3)
        ).astype(ml_dtypes.float8_e4m3)  # [P, K2, 2, TT]
    else:
        xT = np.ascontiguousarray(xr).astype(bf)
    maskf = attention_mask.astype(np.float32)
    mrowinv = np.ascontiguousarray(
        ((1.0 - maskf) * 1e30 + 1e-20).reshape(1, TT)).astype(np.float32)
    # per-key exp bias: ESHIFT, plus MASKP for masked keys (exp -> ~0)
    mb = ESHIFT + MASKP * (1.0 - maskf)
    mbias = np.ascontiguousarray(
        mb.reshape(B, NKT, P).transpose(2, 0, 1)).astype(np.float32)  # [P,B,NKT]

    in_maps = []
    for c in range(NCORES):
        m = _prep_core(c, x, attention_mask, W_attn, b_attn, W_proj)
        m["xT"] = xT
        m["mrowinv"] = mrowinv
        m["mbias"] = mbias
        in_maps.append(m)
    return in_maps


def kernel(x, attention_mask, W_attn, b_attn, W_proj, b_proj):
    b_proj = np.asarray(b_proj, dtype=np.float32)
    nc = _built()
    in_maps = build_in_maps(x, attention_mask, W_attn, b_attn, W_proj)
    res = run_bass_kernel_spmd(nc, in_maps, core_ids=list(range(NCORES)))
    acc = np.zeros((TT, D), dtype=np.float32)
    for c in range(NCORES):
        acc += res.results[c]["out"].astype(np.float32)
    acc += b_proj[None, :]
    return acc.reshape(B, T, D)
